# revision 34
# baseline (speedup 1.0000x reference)
"""Trainium2 Bass kernel for nn_DecoderPp (PointNet++-style 3-level KNN decoder).

Data-parallel over 16 graphs: core g owns graphs 2g, 2g+1. Per level:
- PE computes s' = -d^2 via K=5 matmuls (positions, |p|^2, |q|^2 folded in).
- DVE max8 (+match_replace for k=16) finds the k-th threshold value exactly.
- DVE reciprocal gives negative-space weights 1/s'; a fused DVE
  scalar_tensor_tensor applies the threshold mask and multiply.
- ACT Copy with a per-partition scale AP normalizes rows by -1/sum(w) and
  casts the dense weight matrix to bf16 (only Copy/Identity/Square/Tanh run
  on ACT -- one activation table, no reloads).
- Per-128 xbar DMA transposes feed bf16 aggregation matmuls y^T = xe^T W^T,
  then the MLP runs feature-major on PE with tanh/bias fused into ACT.
Built on Bacc (finalize() legalizes multi-semaphore waits via EVSEM; raw
Bass trips walrus's one-sync-wait-per-instruction limit). Pool runs only
custom ucode ops on this toolchain, so it is used just for SWDGE loads.
"""
import sys
from contextlib import ExitStack

if "/opt/trn_rl_repo" not in sys.path:
    sys.path.insert(0, "/opt/trn_rl_repo")

import numpy as np

import concourse.bass as bass
import concourse.mybir as mybir
from concourse.bacc import Bacc
from concourse.tile import TileContext
from concourse.masks import make_identity

dt = mybir.dt
AF = mybir.ActivationFunctionType
ALU = mybir.AluOpType

N_CORES = 8
GRAPHS_PER_CORE = 2
N3G, N2G, N1G, N0G = 64, 256, 1024, 4096  # per-graph sizes per level

NEG_BIG = -1.0e30
MASK_THRESH = -1.0e29

f32 = dt.float32
bf16 = dt.bfloat16


def _ceil_div(a, b):
    return (a + b - 1) // b


def build_module(debug=False):
    nc = Bacc()

    P = {}

    def param(name, shape, out=False):
        P[name] = nc.declare_dram_parameter(name, list(shape), f32, isOutput=out)

    param("x", (GRAPHS_PER_CORE * N3G, 256))
    param("pos", (GRAPHS_PER_CORE * N3G, 3))
    param("xs2", (GRAPHS_PER_CORE * N2G, 128))
    param("ps2", (GRAPHS_PER_CORE * N2G, 3))
    param("xs1", (GRAPHS_PER_CORE * N1G, 64))
    param("ps1", (GRAPHS_PER_CORE * N1G, 3))
    param("xs0", (GRAPHS_PER_CORE * N0G, 3))
    param("ps0", (GRAPHS_PER_CORE * N0G, 3))
    for nm, shp in [
        ("W3a", (128, 384)), ("b3a", (128,)),
        ("W3b", (128, 128)), ("b3b", (128,)),
        ("W2a", (64, 192)), ("b2a", (64,)),
        ("W2b", (64, 64)), ("b2b", (64,)),
        ("W1a", (64, 67)), ("b1a", (64,)),
        ("W1b", (64, 64)), ("b1b", (64,)),
        ("W1c", (3, 64)), ("b1c", (3,)),
    ]:
        param(nm, shp)
    param("out", (GRAPHS_PER_CORE * N0G, 3), out=True)
    if debug:
        param("dbg_s3", (128, 64), out=True)
        param("dbg_zap3", (128, 64), out=True)
        param("dbg_v16", (128, 8), out=True)
        param("dbg_sw", (128, 1), out=True)
        param("dbg_w3", (128, 64), out=True)
        param("dbg_W3", (128, 64), out=True)
        param("dbg_qT3", (5, 256), out=True)
        param("dbg_pT3", (5, 64), out=True)
        param("dbg_y3", (128, 128), out=True)
        param("dbg_h3T", (128, 256), out=True)
        param("dbg_h2T", (64, 1024), out=True)
        param("dbg_s1", (128, 1024), out=True)
        param("dbg_zap1", (128, 1024), out=True)
        param("dbg_v16b", (128, 16), out=True)
        param("dbg_sw1", (128, 1), out=True)
        param("dbg_W1", (128, 1024), out=True)
        param("dbg_y1", (64, 128), out=True)
        param("dbg_skc1", (3, 128), out=True)

    with TileContext(nc) as tc, ExitStack() as ctx:
        consts = ctx.enter_context(tc.tile_pool(name="consts", bufs=1))
        wpool = ctx.enter_context(tc.tile_pool(name="weights", bufs=1))
        gpool = ctx.enter_context(tc.tile_pool(name="graph", bufs=2))
        tpool = ctx.enter_context(tc.tile_pool(name="tiles", bufs=5))
        npool = ctx.enter_context(tc.tile_pool(name="narrow", bufs=8))
        pspool = ctx.enter_context(tc.tile_pool(name="ps_s", bufs=2, space="PSUM"))
        psy = ctx.enter_context(tc.tile_pool(name="ps_y", bufs=2, space="PSUM"))
        psm = ctx.enter_context(tc.tile_pool(name="ps_mlp", bufs=1, space="PSUM"))
        pst = ctx.enter_context(tc.tile_pool(name="ps_tp", bufs=1, space="PSUM"))

        ident0 = consts.tile([128, 128], f32)
        make_identity(nc, ident0)
        # ACT-written copy: PE transposes read this so their input waits
        # collapse onto the Activation semaphore (walrus LDW 1-wait limit)
        ident = consts.tile([128, 128], f32)
        nc.scalar.activation(ident[:, :], ident0[:, :], AF.Copy)

        # ---- weight prep: transposed chunks + f32 bias columns.
        # DMA'd weights are staged through an ACT copy so the transpose
        # matmuls wait on a single engine (walrus LDW sync-wait limit). ----
        def prep_linear(wname, bname, O, I, splits, wdtype=bf16):
            w_sb = wpool.tile([O, I], f32, tag=f"{wname}_raw")
            nc.gpsimd.dma_start(w_sb[:, :], P[wname].ap())
            chunks = []
            c0 = 0
            for j, cw in enumerate(splits):
                c1 = c0 + cw
                ps_t = psm.tile([128, 128], f32, tag="mlp")
                nc.tensor.transpose(ps_t[:cw, :O], w_sb[:, c0:c1],
                                    ident[:O, :O])
                wt = wpool.tile([cw, O], wdtype, tag=f"{wname}T{j}")
                nc.scalar.activation(wt[:, :], ps_t[:cw, :O], AF.Copy)
                chunks.append((wt, cw))
                c0 = c1
            bcol = wpool.tile([O, 1], f32, tag=f"{bname}col")
            nc.gpsimd.dma_start(bcol[:, :], P[bname].ap())
            return chunks, bcol

        W3aT, b3a = prep_linear("W3a", "b3a", 128, 384, [128, 128, 128])
        W3bT, b3b = prep_linear("W3b", "b3b", 128, 128, [128])
        W2aT, b2a = prep_linear("W2a", "b2a", 64, 192, [128, 64])
        W2bT, b2b = prep_linear("W2b", "b2b", 64, 64, [64])
        W1aT, b1a = prep_linear("W1a", "b1a", 64, 67, [64, 3])
        W1bT, b1b = prep_linear("W1b", "b1b", 64, 64, [64], wdtype=f32)
        W1cT, b1c = prep_linear("W1c", "b1c", 3, 64, [64], wdtype=f32)

        def load_nat_batch(dram, base, n, d, tag):
            """One DMA: dram rows [base:base+n, :d] -> [128, (n//128)*d]."""
            a = n // 128
            t = gpool.tile([128, a * d], f32, tag=tag)
            src_ap = dram.ap()[base : base + n, :].rearrange(
                "(a p) d -> p a d", p=128)
            nc.gpsimd.dma_start(t[:, :], src_ap)
            return t

        def pos5_chunk(pn, rows, scale3, sq_col, one_col, sq_scale, dst, dc0):
            """Build [rows,5] = columns of scaled pos, -|p|^2 (at sq_col,
            scaled), and 1 (at one_col) from preloaded natural pos [rows,3];
            transpose on PE and copy into dst[:, dc0:dc0+rows]."""
            p5 = tpool.tile([128, 5], f32, tag="pos5")
            nc.scalar.activation(p5[:rows, 0:3], pn[:rows, :], AF.Copy,
                                 scale=scale3)
            sqs = tpool.tile([128, 3], f32, tag="possq")
            ppc = npool.tile([128, 1], f32, tag="ppc")
            nc.scalar.activation(sqs[:rows, :], pn[:rows, :], AF.Square,
                                 accum_out=ppc[:rows, :])
            nc.scalar.activation(p5[:rows, sq_col : sq_col + 1], ppc[:rows, :],
                                 AF.Copy, scale=sq_scale)
            # ones column via ACT (keep all p5 writers on one engine)
            nc.scalar.activation(p5[:rows, one_col : one_col + 1],
                                 ppc[:rows, :], AF.Copy, scale=0.0, bias=1.0)
            t_ps = pst.tile([128, 128], f32, tag="tpos")
            nc.tensor.transpose(t_ps[:5, :rows], p5[:rows, 0:5],
                                ident[:rows, :rows])
            nc.scalar.activation(dst[:, dc0 : dc0 + rows], t_ps[:5, :rows],
                                 AF.Copy)

        # ---------------- one interpolation+MLP level ----------------
        def prop_level(g, lvl, ns, nt, k, Cs, xe_chunks, p_dram, q_dram,
                       skip_dram, Ck, mlp, out_tile):
            """mlp: list of (chunks, bcol, tanh?, O, out_dtype)."""
            # p-side [5, ns]: rows = [p^T; -|p|^2; 1] assembled per 128-chunk
            pT = gpool.tile([5, ns], f32, tag=f"pT{lvl}")
            if ns >= 128:
                pnb = load_nat_batch(p_dram, g * ns, ns, 3, f"pnb{lvl}")
            else:
                pnb = gpool.tile([128, 3], f32, tag=f"pnb{lvl}")
                nc.gpsimd.dma_start(pnb[:ns, :],
                                  p_dram.ap()[g * ns : (g + 1) * ns, :])
            qnb = load_nat_batch(q_dram, g * nt, nt, 3, f"qnb{lvl}")
            sknb = None
            if Ck <= 4:
                sknb = load_nat_batch(skip_dram, g * nt, nt, Ck, f"sknb{lvl}")
            for ci in range(_ceil_div(ns, 128)):
                rows = min(128, ns - ci * 128)
                pos5_chunk(pnb[:, 3 * ci : 3 * ci + 3], rows, 1.0, 3, 4, -1.0,
                           pT, ci * 128)

            ns_pad = max(128, ns)
            n_sch = _ceil_div(ns, 128)
            nfc = _ceil_div(Cs, 128)

            for ti in range(nt // 128):
                t0 = ti * 128
                # q lhsT [5,128]: rows = [2q^T; 1; -|q|^2]
                qlhs = tpool.tile([5, 128], f32, tag="qlhs")
                pos5_chunk(qnb[:, 3 * ti : 3 * ti + 3], 128, 2.0, 4, 3, -1.0,
                           qlhs, 0)
                # s' = -d2 : [128, ns] PSUM (K=5 matmul)
                s_ps = pspool.tile([128, 1024], f32, tag="s")
                for h0 in range(0, ns, 512):
                    h1 = min(ns, h0 + 512)
                    nc.tensor.matmul(s_ps[:, h0:h1], qlhs[:, :],
                                     pT[:, h0:h1], start=True, stop=True)
                sv = s_ps[:, :ns]

                # --- selection (values only) ---
                v16 = npool.tile([128, 16], f32, tag="v16")
                nc.vector.max(v16[:, 0:8], sv)
                if k == 16:
                    zap = tpool.tile([128, ns_pad], f32, tag="zap")
                    nc.vector.match_replace(zap[:, :ns], v16[:, 0:8], sv,
                                            NEG_BIG)
                    nc.vector.max(v16[:, 8:16], zap[:, :ns])

                # --- dense weights via reciprocal in negative space ---
                # wneg = 1/s' = -1/d2 < 0; selected iff wneg <= 1/v_k
                wneg = tpool.tile([128, ns_pad], f32, tag="wneg")
                nc.vector.reciprocal(wneg[:, :ns], sv)
                taur = npool.tile([128, 1], f32, tag="taur")
                nc.vector.reciprocal(taur[:, :], v16[:, k - 1 : k])
                Wraw = tpool.tile([128, ns_pad], f32, tag="Wraw")
                swneg = npool.tile([128, 1], f32, tag="swneg")
                nc.vector.scalar_tensor_tensor(
                    Wraw[:, :ns], wneg[:, :ns], taur[:, :], wneg[:, :ns],
                    op0=ALU.is_le, op1=ALU.mult, accum_out=swneg[:, :])
                swrec = npool.tile([128, 1], f32, tag="swrec")
                nc.vector.reciprocal(swrec[:, :], swneg[:, :])
                W = tpool.tile([128, ns_pad], bf16, tag="W")
                if ns < ns_pad:
                    nc.vector.memset(W[:, ns:], 0.0)
                # W = Wraw * (1/sum(Wraw)) -- negatives cancel
                nc.scalar.activation(W[:, :ns], Wraw[:, :ns], AF.Copy,
                                     scale=swrec[:, :])

                if debug and g == 0 and lvl == 1 and ti == 0:
                    stg1 = tpool.tile([128, 1024], f32, tag="dbgstg1")
                    nc.scalar.activation(stg1[:, :], s_ps[:, :], AF.Copy)
                    nc.sync.dma_start(P["dbg_s1"].ap(), stg1[:, :])
                    nc.sync.dma_start(P["dbg_v16b"].ap(), v16[:, :])
                    nc.sync.dma_start(P["dbg_sw1"].ap(), sw[:, :])
                    nc.gpsimd.dma_start(P["dbg_W1"].ap(), W[:, :])
                if debug and g == 0 and lvl == 3 and ti == 0:
                    stg = tpool.tile([128, 64], f32, tag="dbgstg")
                    nc.scalar.activation(stg[:, :], s_ps[:, :64], AF.Copy)
                    nc.sync.dma_start(P["dbg_s3"].ap(), stg[:, :])
                    nc.sync.dma_start(P["dbg_v16"].ap(), v16[:, 0:8])
                    nc.sync.dma_start(P["dbg_sw"].ap(), sw[:, :])
                    nc.sync.dma_start(P["dbg_w3"].ap(), Wraw[:, :64])
                    nc.gpsimd.dma_start(P["dbg_W3"].ap(), W[:, :64])

                # --- transpose W chunks; aggregate y^T = xe^T @ W^T ---
                WT = []
                for j in range(ns_pad // 128):
                    wt = tpool.tile([128, 128], bf16, tag=f"WT{j}")
                    nc.sync.dma_start_transpose(
                        wt[:, :], W[:, j * 128 : (j + 1) * 128])
                    WT.append(wt)

                y_ps = []
                for fc in range(nfc):
                    f0, f1 = fc * 128, min(Cs, (fc + 1) * 128)
                    yp = psy.tile([128, 128], f32, tag="y")
                    for j in range(n_sch):
                        kr = min(128, ns - j * 128)
                        nc.tensor.matmul(yp[: f1 - f0, :],
                                         xe_chunks[j][0][:kr, f0:f1],
                                         WT[j][:kr, :],
                                         start=(j == 0), stop=(j == n_sch - 1))
                    y_ps.append((yp, f1 - f0))

                # --- MLP input chunks: y^T (bf16) + skip^T ---
                in_chunks = []
                for fc, (yp, fw) in enumerate(y_ps):
                    hc = tpool.tile([128, 128], bf16, tag=f"hc{fc}")
                    nc.scalar.activation(hc[:fw, :], yp[:fw, :], AF.Copy)
                    in_chunks.append((hc, fw))
                if Ck <= 4:
                    sk_ps = pst.tile([128, 128], f32, tag="tpos")
                    nc.tensor.transpose(sk_ps[:Ck, :],
                                        sknb[:, Ck * ti : Ck * ti + Ck],
                                        ident[:, :])
                    skc = tpool.tile([Ck, 128], bf16, tag="skc")
                    nc.scalar.activation(skc[:, :], sk_ps[:Ck, :], AF.Copy)
                    in_chunks.append((skc, Ck))
                else:
                    sk_nat = tpool.tile([128, 128], bf16, tag="sknat")
                    base = g * nt
                    nc.gpsimd.dma_start(
                        sk_nat[:, :Ck],
                        skip_dram.ap()[base + t0 : base + t0 + 128, :])
                    if Ck < 128:
                        nc.vector.memset(sk_nat[:, Ck:], 0.0)
                    skc = tpool.tile([128, 128], bf16, tag="skc")
                    nc.sync.dma_start_transpose(skc[:, :], sk_nat[:, :])
                    in_chunks.append((skc, Ck))

                if debug and g == 0 and lvl == 3 and ti == 0:
                    nc.gpsimd.dma_start(P["dbg_y3"].ap(), in_chunks[0][0][:, :])
                if debug and g == 0 and lvl == 1 and ti == 0:
                    nc.gpsimd.dma_start(P["dbg_y1"].ap(), in_chunks[0][0][:64, :])
                    nc.gpsimd.dma_start(P["dbg_skc1"].ap(), in_chunks[1][0][:, :])

                # --- MLP (feature-major) ---
                cur = in_chunks
                for li, (chunks, bcol, tanh, O, odt) in enumerate(mlp):
                    mp = psm.tile([128, 128], f32, tag="mlp")
                    nkc = len(cur)
                    for j, (ct, kr) in enumerate(cur):
                        wt, cw = chunks[j]
                        assert cw == kr, f"l{lvl} mlp{li} c{j}: {cw} != {kr}"
                        nc.tensor.matmul(mp[:O, :], wt[:, :O], ct[:kr, :],
                                         start=(j == 0), stop=(j == nkc - 1))
                    if li == len(mlp) - 1:
                        nc.scalar.activation(out_tile[:O, t0 : t0 + 128],
                                             mp[:O, :], AF.Identity,
                                             bias=bcol[:, :])
                    else:
                        ho = tpool.tile([128, 128], odt, tag=f"ho{li}")
                        nc.scalar.activation(ho[:O, :], mp[:O, :],
                                             AF.Tanh if tanh else AF.Identity,
                                             bias=bcol[:, :])
                        cur = [(ho, O)]

        # ---------------- per-graph pipeline ----------------
        for g in range(GRAPHS_PER_CORE):
            # level 3: x[64,256] -> h3 [256,128]
            xe3f = gpool.tile([64, 256], f32, tag="xe3f")
            nc.gpsimd.dma_start(xe3f[:, :], P["x"].ap()[g * 64 : (g + 1) * 64, :])
            xe3 = gpool.tile([64, 256], bf16, tag="xe3")
            nc.scalar.activation(xe3[:, :], xe3f[:, :], AF.Copy)
            h3T = gpool.tile([128, 256], bf16, tag="h3T")
            prop_level(g, 3, N3G, N2G, 4, 256, [(xe3, 64)], P["pos"], P["ps2"],
                       P["xs2"], 128,
                       [(W3aT, b3a, True, 128, bf16),
                        (W3bT, b3b, False, 128, bf16)], h3T)
            if debug and g == 0:
                nc.gpsimd.dma_start(P["dbg_h3T"].ap(), h3T[:, :])
            h3nat = []
            for j in range(2):
                hn = gpool.tile([128, 128], bf16, tag=f"h3n{j}")
                nc.sync.dma_start_transpose(hn[:, :],
                                            h3T[:, j * 128 : (j + 1) * 128])
                h3nat.append((hn, 128))

            # level 2: h3 [256,128] -> h2 [1024,64]
            h2T = gpool.tile([64, 1024], bf16, tag="h2T")
            prop_level(g, 2, N2G, N1G, 8, 128, h3nat, P["ps2"], P["ps1"],
                       P["xs1"], 64,
                       [(W2aT, b2a, True, 64, bf16),
                        (W2bT, b2b, False, 64, bf16)], h2T)
            if debug and g == 0:
                nc.gpsimd.dma_start(P["dbg_h2T"].ap(), h2T[:, :])
            h2nat = []
            for j in range(8):
                hn = gpool.tile([128, 64], bf16, tag=f"h2n{j}")
                nc.sync.dma_start_transpose(hn[:, :],
                                            h2T[:, j * 128 : (j + 1) * 128])
                h2nat.append((hn, 128))

            # level 1: h2 [1024,64] -> out [4096,3]
            outT = gpool.tile([3, 4096], f32, tag="outT")
            prop_level(g, 1, N1G, N0G, 16, 64, h2nat, P["ps1"], P["ps0"],
                       P["xs0"], 3,
                       [(W1aT, b1a, True, 64, f32),
                        (W1bT, b1b, True, 64, f32),
                        (W1cT, b1c, False, 3, f32)], outT)
            base = g * N0G
            for i in range(3):
                nc.sync.dma_start(P["out"].ap()[base : base + N0G, i],
                                  outT[i : i + 1, :])

    return nc, P


_NC = None


def _get_nc():
    global _NC
    if _NC is None:
        nc = build_module()[0]
        nc.finalize()  # Bacc lowering: EVSEM wait legalization + reg alloc
        _NC = nc
    return _NC


def shard_inputs(inputs):
    f = lambda name: np.ascontiguousarray(np.asarray(inputs[name], np.float32))
    arrs = {
        "x": (f("x"), N3G), "pos": (f("pos"), N3G),
        "xs2": (f("x_skip2"), N2G), "ps2": (f("pos_skip2"), N2G),
        "xs1": (f("x_skip1"), N1G), "ps1": (f("pos_skip1"), N1G),
        "xs0": (f("x_skip0"), N0G), "ps0": (f("pos_skip0"), N0G),
    }
    weights = {k: f(k) for k in ["W3a", "b3a", "W3b", "b3b", "W2a", "b2a",
                                 "W2b", "b2b", "W1a", "b1a", "W1b", "b1b",
                                 "W1c", "b1c"]}
    in_maps = []
    for c in range(N_CORES):
        m = dict(weights)
        for nm, (arr, ng) in arrs.items():
            m[nm] = np.ascontiguousarray(
                arr[2 * c * ng : (2 * c + 2) * ng])
        in_maps.append(m)
    return in_maps


def kernel(**inputs):
    nc = _get_nc()
    in_maps = shard_inputs(inputs)
    from concourse.bass_utils import run_bass_kernel_spmd

    res = run_bass_kernel_spmd(nc, in_maps, list(range(N_CORES)))
    return np.concatenate([np.asarray(r["out"], np.float32)
                           for r in res.results], axis=0)


if __name__ == "__main__":
    nc, _ = build_module()
    print("build ok")
